# revision 1
# baseline (speedup 1.0000x reference)
"""Trainium2 Bass kernel for nn_Conv2d_24833500905755 (3x3 conv, B=32,
C_in=64, C_out=128, 56x56, pad 1, with the reference's mismatched
weight-flatten order).

Math: out[b,co,h,w] = sum_{c,di,dj} xpad[b,c,h+di,w+dj] * Wt[c,di*3+dj,co]
with Wt = K.reshape(576, C_OUT).reshape(C_IN, 9, C_OUT).

Data-parallel: 4 images per NeuronCore, 2 images packed on the
128-partition dim (fp16 matmuls, K=64 contraction per half, concurrent
PE row-group tiles). Raw-bass hand-scheduled engine programs.

Design (40.6us, vs 42.1us baseline; stream is at the fp16 MAC floor):
  - Phase-0 DMAs (x piece-0 + full W) issued PRE-BLOCK on sync/scalar:
    they hit the queues at init-barrier release. No semaphore gating of
    later inputs -- DMA queues drain each engine ring FIFO (round-robin
    between rings), so ring ORDER is the completion priority; a sem gate
    costs ~2.3us (descriptor-gen 0.7 + ring fetch 0.75 + sem lag 0.85).
  - Warm-up bridge: 26 full-width (N=448) junk matmuls ALTERNATING PE
    row-group halves. HAM (the PE clock gate) only counts full-array
    activity as busy; small-N or single-half junk does NOT lift the
    1.2->2.4GHz throttle. Bridge covers entry (~7.5us) to phase-0
    completion (~11.2us); stream then runs warm end to end, and the
    sequencer's ~40-instruction prefetch-refill stall lands mid-bridge.
  - fp16 output staging: PSUM->SBUF copies cast f32->f16 and per-chunk
    output DMAs (28 x 115KB) drain behind the stream; post-stream tail
    is one 8-row chunk instead of a 16/24-row block (rel-err cost of
    fp16 out ~2^-11; measured total 3.5e-4 vs 2e-2 tolerance).
  - Scalar ACT table preloaded by a dummy copy at t~0 (hides 1.3us).
Engine roles:
  Sync:   x pair-0 pieces, half-0 per-chunk output DMAs, final wait
  Scalar: W DMA (pre-block), ACT preload, pair-1 pieces, half-1 copies
          + output DMAs
  Tensor: warm-up bridge + 252 matmuls gated on input-piece/bank-WAR sems
  Vector: half-0 PSUM->SBUF copies
Fixed costs observed (untouchable from the kernel): ~1.2us block entry,
~7.4us walrus epilogue (per-engine semaphore-range clears), ~2.3us DMA
issue-to-sem latency on the critical input path.
"""

from contextlib import ExitStack

import numpy as np

import concourse.bass as bass
import concourse.mybir as mybir
from concourse.bass_utils import run_bass_kernel_spmd

B, C_IN, C_OUT, H = 32, 64, 128, 56
KS = 3
N_CORES = 8
BPC = B // N_CORES
HP = H + 2
RCHUNK = 8
NCHUNK = H // RCHUNK          # 7 chunks/image-pair, 14 global chunks
MM_DT = mybir.dt.float16
NJUNK = 26                    # warm-up bridge matmuls (13 concurrent pairs)
JR = 8                        # full-width junk (N=448), ALTERNATING row-group
                              # halves: HAM only registers "busy" (and lifts
                              # the PE clock gate) when both 64-row groups are
                              # streaming, i.e. full-array activity


def build_nc(mm_dt=MM_DT):
    f32 = mybir.dt.float32
    nc = bass.Bass()
    x_ext = nc.declare_dram_parameter("x", [BPC, C_IN, HP, HP], mm_dt, isOutput=False)
    w_ext = nc.declare_dram_parameter("w", [2 * C_IN, KS * KS, C_OUT], mm_dt, isOutput=False)
    out_ext = nc.declare_dram_parameter("out", [BPC, C_OUT, H, H], mm_dt, isOutput=True)

    n_out_dmas = 2 * 2 * NCHUNK  # pairs * halves * chunks

    with ExitStack() as ctx:
        wt = ctx.enter_context(nc.sbuf_tensor("wt", [2 * C_IN, KS * KS, C_OUT], mm_dt))
        xps = [
            ctx.enter_context(nc.sbuf_tensor(f"xp{p}", [2 * C_IN, HP, HP], mm_dt))
            for p in range(2)
        ]
        # obs[p][half][chunk] - per-chunk fp16 staging
        obs = [
            [
                [
                    ctx.enter_context(
                        nc.sbuf_tensor(f"ob_{p}_{h}_{ci}", [C_OUT, RCHUNK, H], mm_dt)
                    )
                    for ci in range(NCHUNK)
                ]
                for h in range(2)
            ]
            for p in range(2)
        ]
        actp = ctx.enter_context(nc.sbuf_tensor("actp", [C_OUT, 1], f32))
        # banks[slot][half] - 8 PSUM banks
        banks = [
            [
                ctx.enter_context(
                    nc.psum_tensor(f"ps_{s}_{h}", [C_OUT, RCHUNK, H], f32)
                )
                for h in range(2)
            ]
            for s in range(4)
        ]
        s_w = ctx.enter_context(nc.semaphore("s_w"))
        s_x = [ctx.enter_context(nc.semaphore(f"s_x{p}")) for p in range(2)]
        s_xa = ctx.enter_context(nc.semaphore("s_xa"))
        s_mm = ctx.enter_context(nc.semaphore("s_mm"))
        s_cp = ctx.enter_context(nc.semaphore("s_cp"))
        s_cp2 = ctx.enter_context(nc.semaphore("s_cp2"))
        s_out = ctx.enter_context(nc.semaphore("s_out"))

        # Phase-0 DMAs issued pre-block: they execute right at the init
        # barrier release, ~0.3us before in-block instructions. Ring order
        # (FIFO per engine ring, round-robin between rings) gives them
        # completion priority over the later pieces; no sem gating (a gate
        # costs ~2.3us of descriptor-gen + ring-fetch + sem latency).
        src0 = x_ext[0:2].rearrange("b c h w -> (b c) h w")
        nc.sync.dma_start(out=xps[0][:, 0:10, :], in_=src0[:, 0:10, :]).then_inc(s_xa, 16)
        nc.scalar.dma_start(out=wt[:], in_=w_ext[:]).then_inc(s_w, 16)

        with nc.Block() as block:

            @block.sync
            def _(sync: bass.BassEngine):
                src = x_ext[0:2].rearrange("b c h w -> (b c) h w")
                sync.dma_start(out=xps[0][:, 10:34, :], in_=src[:, 10:34, :]).then_inc(s_x[0], 16)
                sync.dma_start(out=xps[0][:, 34:HP, :], in_=src[:, 34:HP, :]).then_inc(s_x[0], 16)
                for p in range(2):
                    dst = out_ext[2 * p : 2 * p + 1].rearrange("b c h w -> (b c) h w")
                    for ci in range(NCHUNK):
                        c = p * NCHUNK + ci
                        h0 = ci * RCHUNK
                        sync.wait_ge(s_cp, c + 1)
                        sync.dma_start(
                            out=dst[:, h0 : h0 + RCHUNK, :], in_=obs[p][0][ci][:]
                        ).then_inc(s_out, 16)
                sync.wait_ge(s_out, 16 * n_out_dmas)

            @block.scalar
            def _(scalar: bass.BassEngine):
                # trigger the ACT-table load now, not before the first copy
                scalar.copy(out=actp[:], in_=actp[:])
                src = x_ext[2:4].rearrange("b c h w -> (b c) h w")
                scalar.dma_start(out=xps[1][:, 0:10, :], in_=src[:, 0:10, :]).then_inc(s_x[1], 16)
                scalar.dma_start(out=xps[1][:, 10:34, :], in_=src[:, 10:34, :]).then_inc(s_x[1], 16)
                scalar.dma_start(out=xps[1][:, 34:HP, :], in_=src[:, 34:HP, :]).then_inc(s_x[1], 16)
                for p in range(2):
                    dst = out_ext[2 * p + 1 : 2 * p + 2].rearrange("b c h w -> (b c) h w")
                    for ci in range(NCHUNK):
                        c = p * NCHUNK + ci
                        h0 = ci * RCHUNK
                        scalar.wait_ge(s_mm, 2 * (c + 1))
                        scalar.copy(
                            out=obs[p][1][ci][:], in_=banks[c % 4][1][:]
                        ).then_inc(s_cp2, 1)
                        scalar.dma_start(
                            out=dst[:, h0 : h0 + RCHUNK, :], in_=obs[p][1][ci][:]
                        ).then_inc(s_out, 16)

            @block.tensor
            def _(tensor: bass.BassEngine):
                # Warm-up bridge: junk matmuls on not-yet-loaded SBUF keep
                # the PE's HAM activity window hot while the first input
                # piece DMA lands. banks[3] is first reused by chunk 3
                # (start=True clears it), well after these complete.
                for wi in range(NJUNK):
                    h = wi % 2
                    c0 = h * C_IN
                    tensor.matmul(
                        out=banks[3][h][:, 0:JR, :],
                        lhsT=wt[c0 : c0 + C_IN, 0, :],
                        rhs=xps[0][c0 : c0 + C_IN, 0:JR, 0:H],
                        start=True,
                        stop=True,
                    )
                tensor.wait_ge(s_w, 16)
                for p in range(2):
                    for ci in range(NCHUNK):
                        c = p * NCHUNK + ci
                        h0 = ci * RCHUNK
                        if p == 0:
                            if ci == 0:
                                tensor.wait_ge(s_xa, 16)  # rows [0,10)
                            elif ci == 1:
                                tensor.wait_ge(s_x[0], 16)  # rows [10,34)
                            elif ci == 4:
                                tensor.wait_ge(s_x[0], 32)  # rows [34,58)
                        else:
                            if ci == 0:
                                tensor.wait_ge(s_x[1], 16)
                            elif ci == 1:
                                tensor.wait_ge(s_x[1], 32)
                            elif ci == 4:
                                tensor.wait_ge(s_x[1], 48)
                        if c >= 4:
                            # WAR: bank slot c%4 last used by chunk c-4
                            tensor.wait_ge(s_cp, c - 3)
                            tensor.wait_ge(s_cp2, c - 3)
                        for k in range(KS * KS):
                            di, dj = divmod(k, KS)
                            last = k == KS * KS - 1
                            for half in range(2):
                                c0 = half * C_IN
                                mm = tensor.matmul(
                                    out=banks[c % 4][half][:],
                                    lhsT=wt[c0 : c0 + C_IN, k, :],
                                    rhs=xps[p][
                                        c0 : c0 + C_IN,
                                        h0 + di : h0 + di + RCHUNK,
                                        dj : dj + H,
                                    ],
                                    start=(k == 0),
                                    stop=last,
                                )
                                if last and half == 1:
                                    mm.then_inc(s_mm, 2)

            @block.vector
            def _(vector: bass.BassEngine):
                for p in range(2):
                    for ci in range(NCHUNK):
                        c = p * NCHUNK + ci
                        vector.wait_ge(s_mm, 2 * (c + 1))
                        vector.tensor_copy(
                            out=obs[p][0][ci][:],
                            in_=banks[c % 4][0][:],
                        ).then_inc(s_cp, 1)

    return nc


def _prep_inputs(x, K, mm_dt=MM_DT):
    np_dt = mybir.dt.np(mm_dt)
    x = np.ascontiguousarray(np.asarray(x, dtype=np.float32))
    K = np.ascontiguousarray(np.asarray(K, dtype=np.float32))
    xpad = np.pad(x, ((0, 0), (0, 0), (1, 1), (1, 1))).astype(np_dt)
    Wt = K.reshape(KS * KS * C_IN, C_OUT).reshape(C_IN, KS * KS, C_OUT)
    Wrep = np.ascontiguousarray(np.concatenate([Wt, Wt], axis=0)).astype(np_dt)
    shards = xpad.reshape(N_CORES, BPC, C_IN, HP, HP)
    return [{"x": np.ascontiguousarray(shards[i]), "w": Wrep} for i in range(N_CORES)]


def run(x, K, trace=False, mm_dt=MM_DT):
    nc = build_nc(mm_dt)
    in_maps = _prep_inputs(x, K, mm_dt)
    res = run_bass_kernel_spmd(nc, in_maps, list(range(N_CORES)), trace=trace)
    out = np.concatenate([res.results[i]["out"] for i in range(N_CORES)], axis=0)
    return out.astype(np.float32), res


def kernel(x, K):
    out, _ = run(x, K, trace=False)
    return out



# revision 2
# speedup vs baseline: 1.0013x; 1.0013x over previous
"""Trainium2 Bass kernel for nn_Conv2d_24833500905755 (3x3 conv, B=32,
C_in=64, C_out=128, 56x56, pad 1, with the reference's mismatched
weight-flatten order).

Math: out[b,co,h,w] = sum_{c,di,dj} xpad[b,c,h+di,w+dj] * Wt[c,di*3+dj,co]
with Wt = K.reshape(576, C_OUT).reshape(C_IN, 9, C_OUT).

Data-parallel: 4 images per NeuronCore, 2 images packed on the
128-partition dim (fp16 matmuls, K=64 contraction per half, concurrent
PE row-group tiles). Raw-bass hand-scheduled engine programs.

v2 layout (from v1 trace analysis, 40.6us baseline):
  - ALL input DMAs pre-block: sync ring carries x pair-0 pieces
    (ring-FIFO gives piece-0 priority), scalar ring carries W first then
    pair-1 pieces. SDMA round-robins between the two rings at packet
    granularity, so W and piece-0 land concurrently (~1.3us of data).
  - ACT preload moved to gpsimd-free slot AFTER desc-gen of inputs
    (v1 ran its table-load DMA at 8.2-9.5us, contending with the
    critical input window on the shared SDMA engines).
  - Junk warm-up bridge pre-block + shortened: bridge only needs to
    cover block-entry to data-ready (~10.5us); real MMs run cold-but-
    useful if HAM (warm at first-MM + ~3.8us) lags data.
  - Final chunk of each pair-1 half split into two 4-row chunks so the
    post-stream tail (copy + desc-gen + transfer + HBM receipt) is paid
    on a 57KB transfer instead of 115KB.
Fixed costs observed (untouchable): ~1.0us block entry, ~7.3us walrus
epilogue (per-engine semaphore-range clears), ~2us DMA first-byte +
receipt latency on the critical input path.
"""

from contextlib import ExitStack

import numpy as np

import concourse.bass as bass
import concourse.mybir as mybir
from concourse.bass_utils import run_bass_kernel_spmd

B, C_IN, C_OUT, H = 32, 64, 128, 56
KS = 3
N_CORES = 8
BPC = B // N_CORES
HP = H + 2
MM_DT = mybir.dt.float16
NJUNK = 18                    # warm-up bridge matmuls (9 concurrent pairs)
JR = 8                        # full-width junk (N=448), ALTERNATING row-group
                              # halves: HAM only registers "busy" (and lifts
                              # the PE clock gate) when both 64-row groups are
                              # streaming, i.e. full-array activity

# per-pair chunk lists: (start_row, n_rows). pair 1 ends with two 4-row
# chunks so the final copy+DMA tail is half-size.
CHUNKS = [
    [(i * 8, 8) for i in range(7)],
    [(i * 8, 8) for i in range(6)] + [(48, 4), (52, 4)],
]
NCH = [len(c) for c in CHUNKS]
CHUNK_OF = [(p, ci) for p in range(2) for ci in range(NCH[p])]
NCHT = len(CHUNK_OF)          # 15 global chunks


def build_nc(mm_dt=MM_DT, njunk=NJUNK):
    f32 = mybir.dt.float32
    nc = bass.Bass()
    x_ext = nc.declare_dram_parameter("x", [BPC, C_IN, HP, HP], mm_dt, isOutput=False)
    w_ext = nc.declare_dram_parameter("w", [2 * C_IN, KS * KS, C_OUT], mm_dt, isOutput=False)
    out_ext = nc.declare_dram_parameter("out", [BPC, C_OUT, H, H], mm_dt, isOutput=True)

    n_out_dmas = 2 * NCHT  # halves * chunks

    with ExitStack() as ctx:
        wt = ctx.enter_context(nc.sbuf_tensor("wt", [2 * C_IN, KS * KS, C_OUT], mm_dt))
        xps = [
            ctx.enter_context(nc.sbuf_tensor(f"xp{p}", [2 * C_IN, HP, HP], mm_dt))
            for p in range(2)
        ]
        # obs[half][chunk] - per-chunk fp16 staging (global chunk index)
        obs = [
            [
                ctx.enter_context(
                    nc.sbuf_tensor(
                        f"ob_{h}_{c}", [C_OUT, CHUNKS[p][ci][1], H], mm_dt
                    )
                )
                for c, (p, ci) in enumerate(CHUNK_OF)
            ]
            for h in range(2)
        ]
        actp = ctx.enter_context(nc.sbuf_tensor("actp", [C_OUT, 1], f32))
        # banks[slot][half] - 8 PSUM banks
        banks = [
            [
                ctx.enter_context(
                    nc.psum_tensor(f"ps_{s}_{h}", [C_OUT, 8, H], f32)
                )
                for h in range(2)
            ]
            for s in range(4)
        ]
        s_w = ctx.enter_context(nc.semaphore("s_w"))
        s_x = [ctx.enter_context(nc.semaphore(f"s_x{p}")) for p in range(2)]
        s_mm = ctx.enter_context(nc.semaphore("s_mm"))
        s_cp = ctx.enter_context(nc.semaphore("s_cp"))
        s_cp2 = ctx.enter_context(nc.semaphore("s_cp2"))
        s_out = ctx.enter_context(nc.semaphore("s_out"))

        # ALL input DMAs pre-block: they execute right at the init barrier
        # release, ~1.0us before in-block instructions. Ring order (FIFO per
        # engine ring, round-robin between rings at packet granularity)
        # gives piece-0 (sync ring head) and W (scalar ring head) completion
        # priority; later pieces drain behind them during the bridge.
        src0 = x_ext[0:2].rearrange("b c h w -> (b c) h w")
        src1 = x_ext[2:4].rearrange("b c h w -> (b c) h w")
        nc.sync.dma_start(out=xps[0][:, 0:10, :], in_=src0[:, 0:10, :]).then_inc(s_x[0], 16)
        nc.scalar.dma_start(out=wt[:], in_=w_ext[:]).then_inc(s_w, 16)
        nc.sync.dma_start(out=xps[0][:, 10:34, :], in_=src0[:, 10:34, :]).then_inc(s_x[0], 16)
        nc.scalar.dma_start(out=xps[1][:, 0:10, :], in_=src1[:, 0:10, :]).then_inc(s_x[1], 16)
        nc.sync.dma_start(out=xps[0][:, 34:HP, :], in_=src0[:, 34:HP, :]).then_inc(s_x[0], 16)
        nc.scalar.dma_start(out=xps[1][:, 10:34, :], in_=src1[:, 10:34, :]).then_inc(s_x[1], 16)
        nc.scalar.dma_start(out=xps[1][:, 34:HP, :], in_=src1[:, 34:HP, :]).then_inc(s_x[1], 16)

        # Warm-up bridge pre-block too: junk matmuls on not-yet-loaded SBUF
        # keep the PE's HAM activity window hot while the phase-0 DMAs land.
        # banks[3] is first reused by chunk 3 (start=True clears it), well
        # after these complete.
        for wi in range(njunk):
            h = wi % 2
            c0 = h * C_IN
            nc.tensor.matmul(
                out=banks[3][h][:, 0:JR, :],
                lhsT=wt[c0 : c0 + C_IN, 0, :],
                rhs=xps[0][c0 : c0 + C_IN, 0:JR, 0:H],
                start=True,
                stop=True,
            )

        with nc.Block() as block:

            @block.sync
            def _(sync: bass.BassEngine):
                for c, (p, ci) in enumerate(CHUNK_OF):
                    h0, rows = CHUNKS[p][ci]
                    dst = out_ext[2 * p : 2 * p + 1].rearrange("b c h w -> (b c) h w")
                    sync.wait_ge(s_cp, c + 1)
                    sync.dma_start(
                        out=dst[:, h0 : h0 + rows, :], in_=obs[0][c][:]
                    ).then_inc(s_out, 16)
                sync.wait_ge(s_out, 16 * n_out_dmas)

            @block.scalar
            def _(scalar: bass.BassEngine):
                # trigger the ACT-table load now: input desc-gen is done
                # (pre-block), first scalar COPY needs the table at ~14us.
                scalar.copy(out=actp[:], in_=actp[:])
                for c, (p, ci) in enumerate(CHUNK_OF):
                    h0, rows = CHUNKS[p][ci]
                    dst = out_ext[2 * p + 1 : 2 * p + 2].rearrange("b c h w -> (b c) h w")
                    scalar.wait_ge(s_mm, 2 * (c + 1))
                    scalar.copy(
                        out=obs[1][c][:], in_=banks[c % 4][1][:, 0:rows, :]
                    ).then_inc(s_cp2, 1)
                    scalar.dma_start(
                        out=dst[:, h0 : h0 + rows, :], in_=obs[1][c][:]
                    ).then_inc(s_out, 16)

            @block.tensor
            def _(tensor: bass.BassEngine):
                tensor.wait_ge(s_w, 16)
                for c, (p, ci) in enumerate(CHUNK_OF):
                    h0, rows = CHUNKS[p][ci]
                    if ci == 0:
                        tensor.wait_ge(s_x[p], 16)  # rows [0,10)
                    elif ci == 1:
                        tensor.wait_ge(s_x[p], 32)  # rows [10,34)
                    elif ci == 4:
                        tensor.wait_ge(s_x[p], 48)  # rows [34,58)
                    if c >= 4:
                        # WAR: bank slot c%4 last used by chunk c-4
                        tensor.wait_ge(s_cp, c - 3)
                        tensor.wait_ge(s_cp2, c - 3)
                    for k in range(KS * KS):
                        di, dj = divmod(k, KS)
                        last = k == KS * KS - 1
                        for half in range(2):
                            c0 = half * C_IN
                            mm = tensor.matmul(
                                out=banks[c % 4][half][:, 0:rows, :],
                                lhsT=wt[c0 : c0 + C_IN, k, :],
                                rhs=xps[p][
                                    c0 : c0 + C_IN,
                                    h0 + di : h0 + di + rows,
                                    dj : dj + H,
                                ],
                                start=(k == 0),
                                stop=last,
                            )
                            if last and half == 1:
                                mm.then_inc(s_mm, 2)

            @block.vector
            def _(vector: bass.BassEngine):
                for c, (p, ci) in enumerate(CHUNK_OF):
                    rows = CHUNKS[p][ci][1]
                    vector.wait_ge(s_mm, 2 * (c + 1))
                    vector.tensor_copy(
                        out=obs[0][c][:],
                        in_=banks[c % 4][0][:, 0:rows, :],
                    ).then_inc(s_cp, 1)

    return nc


def _prep_inputs(x, K, mm_dt=MM_DT):
    np_dt = mybir.dt.np(mm_dt)
    x = np.ascontiguousarray(np.asarray(x, dtype=np.float32))
    K = np.ascontiguousarray(np.asarray(K, dtype=np.float32))
    xpad = np.pad(x, ((0, 0), (0, 0), (1, 1), (1, 1))).astype(np_dt)
    Wt = K.reshape(KS * KS * C_IN, C_OUT).reshape(C_IN, KS * KS, C_OUT)
    Wrep = np.ascontiguousarray(np.concatenate([Wt, Wt], axis=0)).astype(np_dt)
    shards = xpad.reshape(N_CORES, BPC, C_IN, HP, HP)
    return [{"x": np.ascontiguousarray(shards[i]), "w": Wrep} for i in range(N_CORES)]


def run(x, K, trace=False, mm_dt=MM_DT, njunk=NJUNK):
    nc = build_nc(mm_dt, njunk)
    in_maps = _prep_inputs(x, K, mm_dt)
    res = run_bass_kernel_spmd(nc, in_maps, list(range(N_CORES)), trace=trace)
    out = np.concatenate([res.results[i]["out"] for i in range(N_CORES)], axis=0)
    return out.astype(np.float32), res


def kernel(x, K):
    out, _ = run(x, K, trace=False)
    return out
